# revision 1
# baseline (speedup 1.0000x reference)
"""Trainium2 Bass kernel for nn_Lookback: causal running-mean over T.

out[b, t, c] = (1/(t+1)) * sum_{s<=t} x[b, s, c],  x: [8, 4096, 1024] fp32.

Sharding: data-parallel over batch B — core b handles x[b] ([4096, 1024]).

Per-core algorithm (T tiled into 32 blocks of P=128 rows, pipelined as two
16-tile segments so segment 1's load/phase-A overlaps segment 0's phase B):
  Phase A: tile column-sums  totals[j, c] = sum_p x_j[p, c]
           as a PSUM accumulation of matmuls with indicator weights E_j.
  Phase B: out_k = tril128 @ x_k + G_k @ totals
           where G_k[j, p] = [j < k] broadcasts the carry (sum of previous
           tile totals) to all 128 rows.  Both weights are 0/1 matrices.
           totals rows of the not-yet-finished segment are zeros (memset),
           and G_k only weights rows j < k, so segment 0 outputs are exact.
  Scale by d[t] = 1/(t+1) during PSUM->SBUF eviction (per-partition scalar,
  alternating DVE / ACT), then DMA to DRAM.

Matmuls use float32r (fp32 bits, 1 cycle/row at N>=256 vs 4 for fp32).
"""

import sys

import numpy as np

sys.path.insert(0, "/opt/trn_rl_repo")

import concourse.bass as bass
import concourse.mybir as mybir
import concourse.tile as tile
from concourse import bacc
from concourse.bass_utils import run_bass_kernel_spmd

B, T, C = 8, 4096, 1024
P = 128
NT = T // P          # 32 row tiles per core
NSEG = 4
SEG = NT // NSEG     # 16 tiles per segment
CH = 512             # PSUM bank chunk (fp32)
NCH = C // CH
F32 = mybir.dt.float32
F32R = mybir.dt.float32r

_cache = {}


def _consts():
    """Host-precomputed weight matrices (shared by all cores)."""
    # trilT[q, p] = [q <= p]  (lhsT of the lower-triangular ones matrix)
    tril_t = np.tril(np.ones((P, P), np.float32)).T.copy()
    # E_all[:, k*NT:(k+1)*NT] = E_k with E_k[p, m] = [m == k] (global row)
    e_all = np.zeros((P, NT * NT), np.float32)
    for k in range(NT):
        e_all[:, k * NT + k] = 1.0
    # G_all[:, k*P:(k+1)*P] = G_k with G_k[j, p] = [j < k]
    g_all = np.zeros((NT, NT * P), np.float32)
    for k in range(NT):
        g_all[:k, k * P:(k + 1) * P] = 1.0
    # recip[p, k] = 1 / (128*k + p + 1)
    t_idx = np.arange(T, dtype=np.float64).reshape(NT, P).T  # [P, NT]
    recip = (1.0 / (t_idx + 1.0)).astype(np.float32)
    return tril_t, e_all, g_all, recip


def _build():
    nc = bacc.Bacc("TRN2", target_bir_lowering=False, debug=False, num_devices=B)
    x_d = nc.dram_tensor("x", [T, C], F32R, kind="ExternalInput").ap()
    tril_d = nc.dram_tensor("tril_t", [P, P], F32R, kind="ExternalInput").ap()
    e_d = nc.dram_tensor("e_all", [P, NT * NT], F32R, kind="ExternalInput").ap()
    g_d = nc.dram_tensor("g_all", [NT, NT * P], F32R, kind="ExternalInput").ap()
    r_d = nc.dram_tensor("recip", [P, NT], F32, kind="ExternalInput").ap()
    out_d = nc.dram_tensor("out", [T, C], F32, kind="ExternalOutput").ap()

    x_t = x_d.rearrange("(n p) c -> n p c", p=P)      # [NT, P, C]
    out_t = out_d.rearrange("(n p) c -> n p c", p=P)

    with tile.TileContext(nc) as tc:
        with (
            tc.tile_pool(name="const", bufs=1) as cp,
            tc.tile_pool(name="xres", bufs=1) as xp,
            tc.tile_pool(name="tot", bufs=1) as tp,
            tc.tile_pool(name="ev", bufs=4) as ep,
            tc.tile_pool(name="ps", bufs=3, space=bass.MemorySpace.PSUM) as psp,
            tc.tile_pool(name="pt", bufs=1, space=bass.MemorySpace.PSUM) as ptp,
        ):
            tril_s = cp.tile([P, P], F32R)
            e_s = cp.tile([P, NT * NT], F32R)
            g_s = cp.tile([NT, NT * P], F32R)
            r_s = cp.tile([P, NT], F32)
            nc.sync.dma_start(tril_s[:], tril_d)
            nc.sync.dma_start(e_s[:], e_d)
            nc.sync.dma_start(g_s[:], g_d)
            nc.sync.dma_start(r_s[:], r_d)

            xr = xp.tile([P, NT * C], F32R)           # resident input
            tot_list = []

            # PE warm-up burst: ~10us of back-to-back dummy matmuls while
            # the first segment streams in, so the HAM clock gate reaches
            # 8/8 (2.4 GHz) before the real matmul streams start.
            dmy = psp.tile([P, CH], F32, tag="ps")
            for _ in range(40):
                nc.tensor.matmul(dmy[:], tril_s[:], e_s[:, 0:CH],
                                 start=True, stop=True)

            for s in range(NSEG):
                k0, k1 = s * SEG, (s + 1) * SEG
                pt = ptp.tile([NT, C], F32)
                # ---- load + phase A for this segment -----------------
                for k in range(k0, k1):
                    xs = xr[:, k * C:(k + 1) * C]
                    nc.sync.dma_start(xs, x_t[k])
                    for h in range(NCH):
                        sl = slice(h * CH, (h + 1) * CH)
                        nc.tensor.matmul(
                            pt[:, sl],
                            e_s[:, k * NT:(k + 1) * NT],
                            xs[:, sl],
                            start=(k == k0),
                            stop=(k == k1 - 1),
                        )
                # per-segment running totals tile: no WAR against the G
                # matmuls of earlier segments (they read their own tile)
                tot_s = tp.tile([NT, C], F32R, tag=f"tot{s}")
                if s == 0:
                    nc.vector.tensor_copy(tot_s[:], pt[:])
                else:
                    nc.vector.tensor_add(tot_s[:], tot_list[s - 1][:], pt[:])
                tot_list.append(tot_s)

                # ---- phase B + scaled eviction + store ---------------
                for k in range(k0, k1):
                    xs = xr[:, k * C:(k + 1) * C]
                    ps = psp.tile([P, C], F32)
                    # both chunks of the tril matmul first (same weights),
                    # then both chunks of the carry matmul
                    for h in range(NCH):
                        sl = slice(h * CH, (h + 1) * CH)
                        nc.tensor.matmul(
                            ps[:, sl], tril_s[:], xs[:, sl],
                            start=True, stop=(k == 0),
                        )
                    if k > 0:
                        for h in range(NCH):
                            sl = slice(h * CH, (h + 1) * CH)
                            nc.tensor.matmul(
                                ps[:, sl], g_s[:, k * P:(k + 1) * P], tot_s[:, sl],
                                start=False, stop=True,
                            )
                    o = ep.tile([P, C], F32)
                    scale = r_s[:, k:k + 1]
                    if k % 2 == 0:
                        nc.vector.tensor_scalar_mul(o[:], ps[:], scale)
                    else:
                        nc.scalar.activation(
                            o[:], ps[:], mybir.ActivationFunctionType.Copy,
                            scale=scale,
                        )
                    nc.sync.dma_start(out_t[k], o[:])

    nc.compile()
    return nc


def _run(x, trace=False):
    x = np.ascontiguousarray(x, dtype=np.float32)
    assert x.shape == (B, T, C)
    if "nc" not in _cache:
        _cache["nc"] = _build()
        _cache["consts"] = _consts()
    nc = _cache["nc"]
    tril_t, e_all, g_all, recip = _cache["consts"]
    in_maps = [
        {"x": x[b], "tril_t": tril_t, "e_all": e_all, "g_all": g_all, "recip": recip}
        for b in range(B)
    ]
    res = run_bass_kernel_spmd(nc, in_maps, core_ids=list(range(B)), trace=trace)
    out = np.stack([res.results[b]["out"] for b in range(B)])
    return out, res


def kernel(x):
    out, _ = _run(x, trace=False)
    return out



# revision 5
# speedup vs baseline: 1.7726x; 1.7726x over previous
"""Trainium2 Bass kernel for nn_Lookback: causal running-mean over T.

out[b, t, c] = (1/(t+1)) * sum_{s<=t} x[b, s, c],  x: [8, 4096, 1024] fp32.

Sharding: data-parallel over batch B — core b handles x[b] ([4096, 1024]).

The 2e-2 rel-err budget allows bf16 I/O: the host casts x to bf16, the
kernel streams bf16 and writes bf16 out, halving HBM traffic (the
baseline f32 version was DMA-bound at ~137us; bf16 floor is ~47us).

Per-core algorithm (T tiled into 32 blocks of P=128 rows, 4 segments of
8 tiles, pipelined so segment s+1's load overlaps segment s's phase B):
  Phase A: tile column-sums  totals[j, c] = sum_p x_j[p, c]
           as a PSUM accumulation of matmuls with indicator weights E_j.
  Phase B: out_k = tril128 @ x_k + G_k @ totals
           where G_k[j, p] = [j < k] broadcasts the carry (sum of previous
           tile totals) to all 128 rows.  Both weights are 0/1 matrices.
           Emission order [carry(k-1), evict(k-1), tril(k)] keeps >=4 PE
           ops between a tile's carry and the tril that reuses its PSUM
           buffer, so evictions never stall the PE.
  Scale by d[t] = 1/(t+1) during PSUM->SBUF eviction (per-partition scalar,
  alternating DVE / ACT) into bf16 staging tiles, DMA'd out 4 tiles
  (1 MiB) at a time.

x / tril / E matmuls run in bf16 (f32 PSUM accumulation); the carry path
(G @ totals) stays f32r so totals keep full precision.
"""

import sys

import numpy as np

sys.path.insert(0, "/opt/trn_rl_repo")

import ml_dtypes

import concourse.bass as bass
import concourse.mybir as mybir
import concourse.tile as tile
from concourse import bacc
from concourse.bass_utils import run_bass_kernel_spmd

B, T, C = 8, 4096, 1024
P = 128
NT = T // P          # 32 row tiles per core
NSEG = 4
SEG = NT // NSEG     # 8 tiles per segment
CH = 512             # PSUM bank chunk (fp32)
NCH = C // CH
DB = 4               # tiles per DMA batch (1 MiB in bf16)
F32 = mybir.dt.float32
F32R = mybir.dt.float32r
BF16 = mybir.dt.bfloat16

_cache = {}


def _consts():
    """Host-precomputed weight matrices (shared by all cores)."""
    # trilT[q, p] = [q <= p]  (lhsT of the lower-triangular ones matrix)
    tril_t = np.tril(np.ones((P, P), np.float32)).T.copy()
    # E_all[:, k*NT:(k+1)*NT] = E_k with E_k[p, m] = [m == k] (global row)
    e_all = np.zeros((P, NT * NT), np.float32)
    for k in range(NT):
        e_all[:, k * NT + k] = 1.0
    # G_all[:, k*P:(k+1)*P] = G_k with G_k[j, p] = [j < k]
    g_all = np.zeros((NT, NT * P), np.float32)
    for k in range(NT):
        g_all[:k, k * P:(k + 1) * P] = 1.0
    # recip[p, k] = 1 / (128*k + p + 1)
    t_idx = np.arange(T, dtype=np.float64).reshape(NT, P).T  # [P, NT]
    recip = (1.0 / (t_idx + 1.0)).astype(np.float32)
    return (
        tril_t.astype(ml_dtypes.bfloat16),
        e_all.astype(ml_dtypes.bfloat16),
        g_all,
        recip,
    )


def _build():
    nc = bacc.Bacc("TRN2", target_bir_lowering=False, debug=False, num_devices=B)
    x_d = nc.dram_tensor("x", [T, C], BF16, kind="ExternalInput").ap()
    tril_d = nc.dram_tensor("tril_t", [P, P], BF16, kind="ExternalInput").ap()
    e_d = nc.dram_tensor("e_all", [P, NT * NT], BF16, kind="ExternalInput").ap()
    g_d = nc.dram_tensor("g_all", [NT, NT * P], F32R, kind="ExternalInput").ap()
    r_d = nc.dram_tensor("recip", [P, NT], F32, kind="ExternalInput").ap()
    out_d = nc.dram_tensor("out", [T, C], BF16, kind="ExternalOutput").ap()

    x_pnc = x_d.rearrange("(n p) c -> p n c", p=P)      # [P, NT, C]
    out_pnc = out_d.rearrange("(n p) c -> p n c", p=P)  # [P, NT, C]

    with tile.TileContext(nc) as tc:
        with (
            tc.tile_pool(name="const", bufs=1) as cp,
            tc.tile_pool(name="xres", bufs=1) as xp,
            tc.tile_pool(name="tot", bufs=1) as tp,
            tc.tile_pool(name="ev", bufs=1) as ep,
            tc.tile_pool(name="ps", bufs=3, space=bass.MemorySpace.PSUM) as psp,
            tc.tile_pool(name="pt", bufs=1, space=bass.MemorySpace.PSUM) as ptp,
        ):
            tril_s = cp.tile([P, P], BF16)
            e_s = cp.tile([P, NT * NT], BF16)
            g_s = cp.tile([NT, NT * P], F32R)
            r_s = cp.tile([P, NT], F32)
            nc.sync.dma_start(tril_s[:], tril_d)
            nc.sync.dma_start(e_s[:], e_d)
            nc.sync.dma_start(r_s[:], r_d)
            nc.sync.dma_start(g_s[:], g_d)

            xr = xp.tile([P, NT * C], BF16)           # resident input
            # out staging: rotating bf16 buffers of DB tiles each
            ostage = [
                ep.tile([P, DB * C], BF16, tag=f"o{i}", name=f"o{i}")
                for i in range(3)
            ]
            tot_list = []

            # PE warm-up burst while consts + first loads stream in, so the
            # HAM clock gate reaches 8/8 (2.4 GHz) before the real matmuls.
            dmy = psp.tile([P, CH], F32, tag="ps")
            for _ in range(12):
                nc.tensor.matmul(dmy[:], tril_s[:], e_s[:, 0:CH],
                                 start=True, stop=True)

            # batched loads: 1 MiB (DB tiles) per dma_start
            def load_batch(k0):
                dst = xr[:, k0 * C:(k0 + DB) * C].rearrange(
                    "p (n c) -> p n c", n=DB)
                nc.sync.dma_start(dst, x_pnc[:, k0:k0 + DB, :])

            for kk in range(0, SEG, DB):
                load_batch(kk)

            for s in range(NSEG):
                k0, k1 = s * SEG, (s + 1) * SEG
                pt = ptp.tile([NT, C], F32)
                # ---- phase A for this segment ------------------------
                for k in range(k0, k1):
                    xs = xr[:, k * C:(k + 1) * C]
                    for h in range(NCH):
                        sl = slice(h * CH, (h + 1) * CH)
                        nc.tensor.matmul(
                            pt[:, sl],
                            e_s[:, k * NT:(k + 1) * NT],
                            xs[:, sl],
                            start=(k == k0),
                            stop=(k == k1 - 1),
                        )
                # prefetch next segment's input
                if s + 1 < NSEG:
                    for kk in range((s + 1) * SEG, (s + 2) * SEG, DB):
                        load_batch(kk)
                # per-segment running totals tile (f32r SBUF copy of pt)
                tot_s = tp.tile([NT, C], F32R, tag=f"tot{s}")
                if s == 0:
                    nc.vector.tensor_copy(tot_s[:], pt[:])
                else:
                    nc.vector.tensor_add(tot_s[:], tot_list[s - 1][:], pt[:])
                tot_list.append(tot_s)

                # ---- phase B -----------------------------------------
                stage = [None] * SEG

                def tril_mm(k):
                    xs = xr[:, k * C:(k + 1) * C]
                    ps = psp.tile([P, C], F32)
                    stage[k - k0] = ps
                    for h in range(NCH):
                        sl = slice(h * CH, (h + 1) * CH)
                        nc.tensor.matmul(
                            ps[:, sl], tril_s[:], xs[:, sl],
                            start=True, stop=(k == 0),
                        )

                def carry_mm(k):
                    if k == 0:
                        return
                    ps = stage[k - k0]
                    # first tile of a segment only needs rows j < k0, all
                    # final in the previous segment's totals -> no wait on
                    # this segment's DVE add
                    tot = tot_list[s - 1] if (k == k0 and s > 0) else tot_s
                    for h in range(NCH):
                        sl = slice(h * CH, (h + 1) * CH)
                        nc.tensor.matmul(
                            ps[:, sl], g_s[:, k * P:(k + 1) * P],
                            tot[:, sl],
                            start=False, stop=True,
                        )

                def evict(k):
                    ps = stage[k - k0]
                    ob = k // DB
                    o = ostage[ob % len(ostage)]
                    osl = o[:, (k % DB) * C:((k % DB) + 1) * C]
                    scale = r_s[:, k:k + 1]
                    if k % 2 == 0:
                        nc.vector.tensor_scalar_mul(osl, ps[:], scale)
                    else:
                        nc.scalar.activation(
                            osl, ps[:], mybir.ActivationFunctionType.Copy,
                            scale=scale,
                        )
                    if k % DB == DB - 1:
                        src = o[:].rearrange("p (n c) -> p n c", n=DB)
                        nc.sync.dma_start(
                            out_pnc[:, k - DB + 1:k + 1, :], src)

                for k in range(k0, k1 + 1):
                    if k > k0:
                        carry_mm(k - 1)
                        evict(k - 1)
                    if k < k1:
                        tril_mm(k)

    nc.compile()
    return nc


def _run(x, trace=False):
    assert x.shape == (B, T, C)
    xb = np.ascontiguousarray(x).astype(ml_dtypes.bfloat16)
    if "nc" not in _cache:
        _cache["nc"] = _build()
        _cache["consts"] = _consts()
    nc = _cache["nc"]
    tril_t, e_all, g_all, recip = _cache["consts"]
    in_maps = [
        {"x": xb[b], "tril_t": tril_t, "e_all": e_all, "g_all": g_all,
         "recip": recip}
        for b in range(B)
    ]
    res = run_bass_kernel_spmd(nc, in_maps, core_ids=list(range(B)), trace=trace)
    out = np.stack([res.results[b]["out"].astype(np.float32) for b in range(B)])
    return out, res


def kernel(x):
    out, _ = _run(x, trace=False)
    return out


# revision 6
# speedup vs baseline: 1.8577x; 1.0480x over previous
"""Trainium2 Bass kernel for nn_Lookback: causal running-mean over T.

out[b, t, c] = (1/(t+1)) * sum_{s<=t} x[b, s, c],  x: [8, 4096, 1024] fp32.

Sharding: data-parallel over batch B — core b handles x[b] ([4096, 1024]).

The 2e-2 rel-err budget allows bf16 I/O: the host casts x to bf16, the
kernel streams bf16 and writes bf16 out, halving HBM traffic vs the f32
baseline.  DRAM buffers are laid out partition-major ([P, NT*C]) so every
DMA is per-partition contiguous (128 large descriptors / transfer), and
loads issue from the SP HWDGE ring while stores issue from the ACT ring.

Per-core algorithm (T tiled into 32 blocks of P=128 rows, 4 segments of
8 tiles, pipelined so segment s+1's load overlaps segment s's phase B):
  Phase A: tile column-sums  totals[j, c] = sum_p x_j[p, c]
           as a PSUM accumulation of matmuls with indicator weights E_j.
  Phase B: out_k = tril128 @ x_k + G_k @ totals
           where G_k[j, p] = [j < k] broadcasts the carry (sum of previous
           tile totals) to all 128 rows.  Both weights are 0/1 matrices.
           Emission order [carry(k-1), evict(k-1), tril(k)] keeps ~1.8us
           of PE work between a tile's carry and the tril that reuses its
           PSUM buffer, so evictions never stall the PE.
  Scale by d[t] = 1/(t+1) during PSUM->SBUF eviction (per-partition scalar,
  alternating DVE / ACT) into bf16 staging tiles, DMA'd out 4 tiles
  (1 MiB) at a time.

The whole matmul path is bf16 (f32 PSUM accumulation): an all-bf16 carry
(totals cast to bf16; exact 0/1 G weights) measures ~216 ns/matmul vs
~330 ns for the f32r path.  The f32 running totals live in SBUF, updated
per segment on the DVE.
"""

import sys

import numpy as np

sys.path.insert(0, "/opt/trn_rl_repo")

import ml_dtypes

import concourse.bass as bass
import concourse.mybir as mybir
import concourse.tile as tile
from concourse import bacc
from concourse.bass_utils import run_bass_kernel_spmd

B, T, C = 8, 4096, 1024
P = 128
NT = T // P          # 32 row tiles per core
NSEG = 4
SEG = NT // NSEG     # 8 tiles per segment
CH = 512             # PSUM bank chunk (fp32)
NCH = C // CH
DB = 4               # tiles per DMA batch (1 MiB in bf16)
F32 = mybir.dt.float32
BF16 = mybir.dt.bfloat16

_cache = {}


def _consts():
    """Host-precomputed weight matrices (shared by all cores)."""
    # trilT[q, p] = [q <= p]  (lhsT of the lower-triangular ones matrix)
    tril_t = np.tril(np.ones((P, P), np.float32)).T.copy()
    # E_all[:, k*NT:(k+1)*NT] = E_k with E_k[p, m] = [m == k] (global row)
    e_all = np.zeros((P, NT * NT), np.float32)
    for k in range(NT):
        e_all[:, k * NT + k] = 1.0
    # G_all[:, k*P:(k+1)*P] = G_k with G_k[j, p] = [j < k]
    g_all = np.zeros((NT, NT * P), np.float32)
    for k in range(NT):
        g_all[:k, k * P:(k + 1) * P] = 1.0
    # recip[p, k] = 1 / (128*k + p + 1)
    t_idx = np.arange(T, dtype=np.float64).reshape(NT, P).T  # [P, NT]
    recip = (1.0 / (t_idx + 1.0)).astype(np.float32)
    bf = ml_dtypes.bfloat16
    return tril_t.astype(bf), e_all.astype(bf), g_all.astype(bf), recip


def _build():
    nc = bacc.Bacc("TRN2", target_bir_lowering=False, debug=False, num_devices=B)
    # partition-major layouts: element (k, p, c) lives at [p, k*C + c]
    x_d = nc.dram_tensor("x", [P, NT * C], BF16, kind="ExternalInput").ap()
    tril_d = nc.dram_tensor("tril_t", [P, P], BF16, kind="ExternalInput").ap()
    e_d = nc.dram_tensor("e_all", [P, NT * NT], BF16, kind="ExternalInput").ap()
    g_d = nc.dram_tensor("g_all", [NT, NT * P], BF16, kind="ExternalInput").ap()
    r_d = nc.dram_tensor("recip", [P, NT], F32, kind="ExternalInput").ap()
    out_d = nc.dram_tensor("out", [P, NT * C], BF16, kind="ExternalOutput").ap()

    with tile.TileContext(nc) as tc:
        with (
            tc.tile_pool(name="const", bufs=1) as cp,
            tc.tile_pool(name="xres", bufs=1) as xp,
            tc.tile_pool(name="tot", bufs=1) as tp,
            tc.tile_pool(name="ev", bufs=1) as ep,
            tc.tile_pool(name="ps", bufs=3, space=bass.MemorySpace.PSUM) as psp,
            tc.tile_pool(name="pt", bufs=1, space=bass.MemorySpace.PSUM) as ptp,
        ):
            tril_s = cp.tile([P, P], BF16)
            e_s = cp.tile([P, NT * NT], BF16)
            g_s = cp.tile([NT, NT * P], BF16)
            r_s = cp.tile([P, NT], F32)
            # small consts first on the SP ring (gate the warmup), then the
            # x loads; g/recip go on the ACT ring (stores come much later)
            nc.sync.dma_start(tril_s[:], tril_d)
            nc.sync.dma_start(e_s[:], e_d)
            nc.scalar.dma_start(g_s[:], g_d)
            nc.scalar.dma_start(r_s[:], r_d)

            xr = xp.tile([P, NT * C], BF16)           # resident input
            # out staging: rotating bf16 buffers of DB tiles each
            ostage = [
                ep.tile([P, DB * C], BF16, tag=f"o{i}", name=f"o{i}")
                for i in range(3)
            ]
            cum_list = []

            # PE warm-up burst while consts + first loads stream in, so the
            # HAM clock gate reaches 8/8 (2.4 GHz) before the real matmuls.
            dmy = psp.tile([P, CH], F32, tag="ps")
            for _ in range(12):
                nc.tensor.matmul(dmy[:], tril_s[:], e_s[:, 0:CH],
                                 start=True, stop=True)

            def load_batch(k0):
                sl = slice(k0 * C, (k0 + DB) * C)
                nc.sync.dma_start(xr[:, sl], x_d[:, sl])

            for kk in range(0, SEG, DB):
                load_batch(kk)

            for s in range(NSEG):
                k0, k1 = s * SEG, (s + 1) * SEG
                pt = ptp.tile([NT, C], F32)
                # ---- phase A (tile 31's column sum is never used) -----
                ka_end = k1 - 1 if s == NSEG - 1 else k1
                for k in range(k0, ka_end):
                    xs = xr[:, k * C:(k + 1) * C]
                    for h in range(NCH):
                        sl = slice(h * CH, (h + 1) * CH)
                        nc.tensor.matmul(
                            pt[:, sl],
                            e_s[:, k * NT:(k + 1) * NT],
                            xs[:, sl],
                            start=(k == k0),
                            stop=(k == ka_end - 1),
                        )
                # prefetch next segment's input
                if s + 1 < NSEG:
                    for kk in range((s + 1) * SEG, (s + 2) * SEG, DB):
                        load_batch(kk)
                # running totals: f32 accumulator + bf16 copy for the carry
                cum_s = tp.tile([NT, C], F32, tag=f"cum{s}", name=f"cum{s}")
                totb_s = tp.tile([NT, C], BF16, tag=f"tb{s}", name=f"tb{s}")
                if s == 0:
                    nc.vector.tensor_copy(cum_s[:], pt[:])
                else:
                    nc.vector.tensor_add(cum_s[:], cum_list[s - 1][:], pt[:])
                nc.vector.tensor_copy(totb_s[:], cum_s[:])
                cum_list.append(cum_s)

                # ---- phase B -----------------------------------------
                stage = [None] * SEG

                def tril_mm(k):
                    xs = xr[:, k * C:(k + 1) * C]
                    ps = psp.tile([P, C], F32)
                    stage[k - k0] = ps
                    for h in range(NCH):
                        sl = slice(h * CH, (h + 1) * CH)
                        nc.tensor.matmul(
                            ps[:, sl], tril_s[:], xs[:, sl],
                            start=True, stop=(k == 0),
                        )

                def carry_mm(k):
                    if k == 0:
                        return
                    ps = stage[k - k0]
                    for h in range(NCH):
                        sl = slice(h * CH, (h + 1) * CH)
                        nc.tensor.matmul(
                            ps[:, sl], g_s[:, k * P:(k + 1) * P],
                            totb_s[:, sl],
                            start=False, stop=True,
                        )

                def evict(k):
                    ps = stage[k - k0]
                    ob = k // DB
                    o = ostage[ob % len(ostage)]
                    osl = o[:, (k % DB) * C:((k % DB) + 1) * C]
                    scale = r_s[:, k:k + 1]
                    if k % 2 == 0:
                        nc.vector.tensor_scalar_mul(osl, ps[:], scale)
                    else:
                        nc.scalar.activation(
                            osl, ps[:], mybir.ActivationFunctionType.Copy,
                            scale=scale,
                        )
                    if k % DB == DB - 1:
                        sl = slice((k - DB + 1) * C, (k + 1) * C)
                        nc.scalar.dma_start(out_d[:, sl], o[:])

                for k in range(k0, k1 + 1):
                    if k > k0:
                        carry_mm(k - 1)
                        evict(k - 1)
                    if k < k1:
                        tril_mm(k)

    nc.compile()
    return nc


def _run(x, trace=False):
    assert x.shape == (B, T, C)
    # partition-major bf16 layout: [B, P, NT*C]
    xb = (
        np.ascontiguousarray(x)
        .astype(ml_dtypes.bfloat16)
        .reshape(B, NT, P, C)
        .transpose(0, 2, 1, 3)
        .reshape(B, P, NT * C)
        .copy()
    )
    if "nc" not in _cache:
        _cache["nc"] = _build()
        _cache["consts"] = _consts()
    nc = _cache["nc"]
    tril_t, e_all, g_all, recip = _cache["consts"]
    in_maps = [
        {"x": xb[b], "tril_t": tril_t, "e_all": e_all, "g_all": g_all,
         "recip": recip}
        for b in range(B)
    ]
    res = run_bass_kernel_spmd(nc, in_maps, core_ids=list(range(B)), trace=trace)
    out = np.stack([
        res.results[b]["out"]
        .astype(np.float32)
        .reshape(P, NT, C)
        .transpose(1, 0, 2)
        .reshape(T, C)
        for b in range(B)
    ])
    return out, res


def kernel(x):
    out, _ = _run(x, trace=False)
    return out


# revision 7
# speedup vs baseline: 1.9780x; 1.0648x over previous
"""Trainium2 Bass kernel for nn_Lookback: causal running-mean over T.

out[b, t, c] = (1/(t+1)) * sum_{s<=t} x[b, s, c],  x: [8, 4096, 1024] fp32.

Sharding: data-parallel over batch B — core b handles x[b] ([4096, 1024]).

The 2e-2 rel-err budget allows bf16 I/O: the host casts x to bf16, the
kernel streams bf16 and writes bf16 out, halving HBM traffic vs the f32
baseline.  DRAM buffers are laid out partition-major ([P, NT*C]) so every
DMA is per-partition contiguous (128 large descriptors / transfer).
Loads issue on the SP HWDGE ring, stores on the GPSIMD SWDGE ring, so
neither blocks the other (per-ring FIFO) and the ACT engine is free for
evictions.

Per-core algorithm (T tiled into 32 blocks of P=128 rows, 4 segments of
8 tiles, pipelined so segment s+1's load overlaps segment s's phase B):
  Phase A: tile column-sums  totals[j, c] = sum_p x_j[p, c]
           as a PSUM accumulation of matmuls with indicator weights E_j.
  Phase B: out_k = tril128 @ x_k + G_k @ totals, processed in PAIRS:
           the two carry matmuls of a pair run concurrently on PE row
           strips 0 / 32 (K=32 row tiling; G_k staged at base partition
           32*(k%2), totals replicated to [64, C] bf16).
  Scale by d[t] = 1/(t+1) during PSUM->SBUF eviction, split per tile
  across DVE (cols 0:512) and ACT (cols 512:1024) so eviction latency
  (~0.64us) stays under the PE work between PSUM-buffer reuses.
  PE warm-up runs on a memset tile so it needs no DMA and the HAM clock
  gate reaches 8/8 before the first real matmul.

The whole matmul path is bf16 (f32 PSUM accumulation); the f32 running
totals live in SBUF, updated per segment on the DVE.
"""

import sys

import numpy as np

sys.path.insert(0, "/opt/trn_rl_repo")

import ml_dtypes

import concourse.bass as bass
import concourse.mybir as mybir
import concourse.tile as tile
from concourse import bacc
from concourse.bass_utils import run_bass_kernel_spmd

B, T, C = 8, 4096, 1024
P = 128
NT = T // P          # 32 row tiles per core
NSEG = 4
SEG = NT // NSEG     # 8 tiles per segment
CH = 512             # PSUM bank chunk (fp32)
NCH = C // CH
DB = 4               # tiles per store batch (1 MiB in bf16)
F32 = mybir.dt.float32
BF16 = mybir.dt.bfloat16

_cache = {}


def _consts():
    """Host-precomputed weight matrices (shared by all cores)."""
    # trilT[q, p] = [q <= p]  (lhsT of the lower-triangular ones matrix)
    tril_t = np.tril(np.ones((P, P), np.float32)).T.copy()
    # E_all[:, k*NT:(k+1)*NT] = E_k with E_k[p, m] = [m == k] (global row)
    e_all = np.zeros((P, NT * NT), np.float32)
    for k in range(NT):
        e_all[:, k * NT + k] = 1.0
    wconst = np.concatenate([tril_t, e_all], axis=1)  # [P, P + NT*NT]
    # G2: carry weights for tile k at base partition 32*(k%2)
    g2 = np.zeros((2 * NT, NT * P), np.float32)
    for k in range(NT):
        off = NT * (k % 2)
        g2[off:off + k, k * P:(k + 1) * P] = 1.0
    # recip[p, k] = 1 / (128*k + p + 1)
    t_idx = np.arange(T, dtype=np.float64).reshape(NT, P).T  # [P, NT]
    recip = (1.0 / (t_idx + 1.0)).astype(np.float32)
    bf = ml_dtypes.bfloat16
    return wconst.astype(bf), g2.astype(bf), recip


def _build():
    nc = bacc.Bacc("TRN2", target_bir_lowering=False, debug=False, num_devices=B)
    # partition-major layouts: element (k, p, c) lives at [p, k*C + c]
    x_d = nc.dram_tensor("x", [P, NT * C], BF16, kind="ExternalInput").ap()
    w_d = nc.dram_tensor("wconst", [P, P + NT * NT], BF16,
                         kind="ExternalInput").ap()
    g_d = nc.dram_tensor("g2", [2 * NT, NT * P], BF16, kind="ExternalInput").ap()
    r_d = nc.dram_tensor("recip", [P, NT], F32, kind="ExternalInput").ap()
    out_d = nc.dram_tensor("out", [P, NT * C], BF16, kind="ExternalOutput").ap()

    with tile.TileContext(nc) as tc:
        with (
            tc.tile_pool(name="const", bufs=1) as cp,
            tc.tile_pool(name="xres", bufs=1) as xp,
            tc.tile_pool(name="tot", bufs=1) as tp,
            tc.tile_pool(name="ev", bufs=1) as ep,
            tc.tile_pool(name="ps", bufs=3, space=bass.MemorySpace.PSUM) as psp,
            tc.tile_pool(name="pt", bufs=1, space=bass.MemorySpace.PSUM) as ptp,
        ):
            w_s = cp.tile([P, P + NT * NT], BF16)
            g_s = cp.tile([2 * NT, NT * P], BF16)
            r_s = cp.tile([P, NT], F32)
            warm = cp.tile([P, P + CH], BF16)
            nc.sync.dma_start(w_s[:], w_d)
            nc.scalar.dma_start(g_s[:], g_d)
            nc.scalar.dma_start(r_s[:], r_d)
            tril_s = w_s[:, 0:P]
            e_s = w_s[:, P:P + NT * NT]

            xr = xp.tile([P, NT * C], BF16)           # resident input
            # out staging: rotating bf16 buffers of DB tiles each
            ostage = [
                ep.tile([P, DB * C], BF16, tag=f"o{i}", name=f"o{i}")
                for i in range(3)
            ]
            cum_list, totb_list = [], []

            # PE warm-up on a memset tile — no DMA dependency, so it runs
            # during the preamble and the HAM clock gate is already 8/8
            # (2.4 GHz) when the first real matmul issues.
            nc.vector.memset(warm[:], 0)
            dmy = psp.tile([P, CH], F32, tag="ps")
            for _ in range(10):
                nc.tensor.matmul(dmy[:], warm[:, 0:P], warm[:, P:P + CH],
                                 start=True, stop=True)

            def load_batch(k0):
                sl = slice(k0 * C, (k0 + DB) * C)
                nc.sync.dma_start(xr[:, sl], x_d[:, sl])

            for kk in range(0, SEG, DB):
                load_batch(kk)

            for s in range(NSEG):
                k0, k1 = s * SEG, (s + 1) * SEG
                pt = ptp.tile([NT, C], F32)
                # ---- phase A (tile 31's column sum is never used) -----
                ka_end = k1 - 1 if s == NSEG - 1 else k1
                for k in range(k0, ka_end):
                    xs = xr[:, k * C:(k + 1) * C]
                    for h in range(NCH):
                        sl = slice(h * CH, (h + 1) * CH)
                        nc.tensor.matmul(
                            pt[:, sl],
                            e_s[:, k * NT:(k + 1) * NT],
                            xs[:, sl],
                            start=(k == k0),
                            stop=(k == ka_end - 1),
                        )
                # prefetch next segment's input
                if s + 1 < NSEG:
                    for kk in range((s + 1) * SEG, (s + 2) * SEG, DB):
                        load_batch(kk)
                # running totals: f32 accumulator + bf16 replicas at base
                # partitions 0 and 32 (for the two carry row strips).
                # Strip-32 copy goes first: the first pair's second tile
                # (strip 32) is the earliest consumer.
                cum_s = tp.tile([NT, C], F32, tag=f"cum{s}", name=f"cum{s}")
                totb_s = tp.tile([2 * NT, C], BF16, tag=f"tb{s}", name=f"tb{s}")
                if s == 0:
                    nc.vector.tensor_copy(cum_s[:], pt[:])
                else:
                    nc.vector.tensor_add(cum_s[:], cum_list[s - 1][:], pt[:])
                nc.vector.tensor_copy(totb_s[NT:2 * NT, :], cum_s[:])
                nc.vector.tensor_copy(totb_s[0:NT, :], cum_s[:])
                cum_list.append(cum_s)
                totb_list.append(totb_s)

                # ---- phase B: pairs (a, b); carries of a pair run
                # concurrently on PE row strips 0 / 32 ------------------
                stage = [None] * SEG

                def tril_mm(k):
                    xs = xr[:, k * C:(k + 1) * C]
                    ps = psp.tile([P, C], F32)
                    stage[k - k0] = ps
                    for h in range(NCH):
                        sl = slice(h * CH, (h + 1) * CH)
                        nc.tensor.matmul(
                            ps[:, sl], tril_s[:], xs[:, sl],
                            start=True, stop=(k == 0),
                        )

                def carry_wave(a, b):
                    for h in range(NCH):
                        sl = slice(h * CH, (h + 1) * CH)
                        for k in (a, b):
                            if k == 0:
                                continue
                            # first tile of a segment only needs rows
                            # j < k0, final in the previous totals
                            tb = (totb_list[s - 1]
                                  if (k == k0 and s > 0) else totb_s)
                            off = NT * (k % 2)
                            nc.tensor.matmul(
                                stage[k - k0][:, sl],
                                g_s[off:off + NT, k * P:(k + 1) * P],
                                tb[off:off + NT, sl],
                                start=False, stop=True,
                            )

                def evict(k):
                    ps = stage[k - k0]
                    ob = k // DB
                    o = ostage[ob % len(ostage)]
                    osl = o[:, (k % DB) * C:((k % DB) + 1) * C]
                    scale = r_s[:, k:k + 1]
                    nc.vector.tensor_scalar_mul(
                        osl[:, 0:CH], ps[:, 0:CH], scale)
                    nc.scalar.activation(
                        osl[:, CH:C], ps[:, CH:C],
                        mybir.ActivationFunctionType.Copy, scale=scale,
                    )
                    # stores: 1 MiB batches; the final batch ships as two
                    # 0.5 MiB halves so the tail drains fast
                    last = (s == NSEG - 1) and (k >= NT - DB)
                    if last:
                        if k % 2 == 1:
                            h0 = (k % DB) - 1
                            sl = slice((k - 1) * C, (k + 1) * C)
                            nc.gpsimd.dma_start(
                                out_d[:, sl], o[:, h0 * C:(h0 + 2) * C])
                    elif k % DB == DB - 1:
                        sl = slice((k - DB + 1) * C, (k + 1) * C)
                        nc.gpsimd.dma_start(out_d[:, sl], o[:])

                for a in range(k0, k1, 2):
                    b = a + 1
                    tril_mm(a)
                    tril_mm(b)
                    carry_wave(a, b)
                    evict(a)
                    evict(b)

    nc.compile()
    return nc


def _run(x, trace=False):
    assert x.shape == (B, T, C)
    # partition-major bf16 layout: [B, P, NT*C]
    xb = (
        np.ascontiguousarray(x)
        .astype(ml_dtypes.bfloat16)
        .reshape(B, NT, P, C)
        .transpose(0, 2, 1, 3)
        .reshape(B, P, NT * C)
        .copy()
    )
    if "nc" not in _cache:
        _cache["nc"] = _build()
        _cache["consts"] = _consts()
    nc = _cache["nc"]
    wconst, g2, recip = _cache["consts"]
    in_maps = [
        {"x": xb[b], "wconst": wconst, "g2": g2, "recip": recip}
        for b in range(B)
    ]
    res = run_bass_kernel_spmd(nc, in_maps, core_ids=list(range(B)), trace=trace)
    out = np.stack([
        res.results[b]["out"]
        .astype(np.float32)
        .reshape(P, NT, C)
        .transpose(1, 0, 2)
        .reshape(T, C)
        for b in range(B)
    ])
    return out, res


def kernel(x):
    out, _ = _run(x, trace=False)
    return out
